# revision 41
# baseline (speedup 1.0000x reference)
"""MoE FFN (grouped sigmoid top-k routing + shared expert) on 8 TRN2 NeuronCores.

Strategy: expert-parallel with SPARSE token dispatch. Routing is computed on
the host (it determines the sharding itself — this harness's stand-in for the
"all-to-all token dispatch after routing" in the sharding hint): each core
gets 2 of 16 routed experts and receives only the tokens routed to them
(gathered + padded to a per-call capacity CAP per expert), plus 1/8 of the
shared expert (sharded along its hidden dim HS) over all tokens. All device
matmuls run in bf16 (rel-err budget 2e-2; bf16 lands ~5e-3), halving DMA and
SBUF versus fp32r at the same PE rate. Each core writes a dense shared-expert
partial [C, S] and its two experts' gathered outputs [C, CAP] (un-weighted);
the host applies the combine weights, scatter-adds, reduces over cores, and
transposes back.

Per-expert capacities are sized to the call's actual loads (slot 0 takes
the 8 most-loaded experts, slot 1 the rest, so slot 1's capacity and its
padding are smaller); no tokens are ever dropped, and the compiled kernel
is cached per capacity pair. Device work per core: (cap0+cap1)*3*C*H
(routed) + 3*S*C*HS/8 (shared) MACs ~ 2.4 G MACs -> ~62 us PE at 2.4 GHz
bf16, vs the dense-dispatch baseline's ~166 us.
"""

import numpy as np
import ml_dtypes

import concourse.bacc as bacc
import concourse.mybir as mybir
from concourse import tile
from concourse.bass_utils import run_bass_kernel_spmd

F32 = mybir.dt.float32
BF = mybir.dt.bfloat16
AF = mybir.ActivationFunctionType
BF_NP = ml_dtypes.bfloat16

# problem shapes (hardcoded; kernel.py must be self-contained)
B, T, C, H, HS = 2, 1024, 1024, 256, 2048
E, G, EPG = 16, 4, 4
TOPK, TOPK_GROUP = 4, 2
PER_GROUP_K = TOPK // TOPK_GROUP
NCORES = 8
S = B * T                  # 2048 tokens
EPC = E // NCORES          # 2 experts per core
HSL = HS // NCORES         # 256 shared-hidden rows per core
KC = C // 128              # 8 contraction chunks
NHC = H // 128             # 2 h chunks (same for HSL)
NSC = S // 512             # 4 moving (token) chunks of 512
NCC = C // 128             # 8 output-row chunks


def build(caps):
    cap0, cap1 = caps
    tcap = cap0 + cap1
    nc = bacc.Bacc(
        "TRN2",
        target_bir_lowering=False,
        debug=False,
        enable_asserts=True,
        num_devices=NCORES,
    )
    # ---- DRAM I/O (per core), all bf16, pre-packed [128, ...] on host ----
    xt_d = nc.declare_dram_parameter("xt", [128, KC * S], BF, isOutput=False)
    xg_d = nc.declare_dram_parameter("xg", [128, KC * tcap], BF,
                                     isOutput=False)
    wgu_d = nc.declare_dram_parameter("wgu", [128, EPC * 2 * KC * H], BF,
                                      isOutput=False)
    wdn_d = nc.declare_dram_parameter("wdn", [128, EPC * NHC * C], BF,
                                      isOutput=False)
    wsgu_d = nc.declare_dram_parameter("wsgu", [128, 2 * KC * HSL], BF,
                                       isOutput=False)
    wsdn_d = nc.declare_dram_parameter("wsdn", [128, NHC * C], BF,
                                       isOutput=False)
    outS_d = nc.declare_dram_parameter("outS", [C, S], BF, isOutput=True)
    outR_d = nc.declare_dram_parameter("outR", [C, tcap], BF,
                                       isOutput=True)

    with tile.TileContext(nc) as tc:
        _emit(nc, tc, caps, xt_d, xg_d, wgu_d, wdn_d, wsgu_d, wsdn_d,
              outS_d, outR_d)
    nc.finalize()
    return nc


def _emit(nc, tc, caps, xt_d, xg_d, wgu_d, wdn_d, wsgu_d, wsdn_d,
          outS_d, outR_d):
    # per-slot moving-chunk lists (each chunk must fit a 512-fp32 psum bank)
    xoff = [0, caps[0]]
    tcap = caps[0] + caps[1]
    chunks = []
    for cp in caps:
        n = -(-cp // 512)          # pieces of <=512 columns, even widths
        base = min(512, -(-cp // n) + (-(-cp // n)) % 2)
        pieces, off = [], 0
        while off < cp:
            w = min(base, cp - off)
            pieces.append((off, w))
            off += w
        chunks.append(pieces)
    res = tc.alloc_tile_pool(name="res", bufs=1)
    xg = res.tile([128, KC * tcap], BF)
    wgu = res.tile([128, EPC * 2 * KC * H], BF)
    wdn = res.tile([128, EPC * NHC * C], BF)
    xt = res.tile([128, KC * S], BF)
    wsgu = res.tile([128, 2 * KC * HSL], BF)
    wsdn = res.tile([128, NHC * C], BF)
    hS = res.tile([128, NHC * S], BF)
    hR = [res.tile([128, NHC * caps[j]], BF, name=f"hR{j}")
          for j in range(EPC)]

    xgv = xg.rearrange("p (k t) -> p k t", k=KC)
    wguv = wgu.rearrange("p (j r k h) -> p j r k h", j=EPC, r=2, k=KC)
    wdnv = wdn.rearrange("p (j hk c) -> p j hk c", j=EPC, hk=NHC)
    xtv = xt.rearrange("p (k s) -> p k s", k=KC)
    wsguv = wsgu.rearrange("p (r k h) -> p r k h", r=2, k=KC)
    wsdnv = wsdn.rearrange("p (hk c) -> p hk c", hk=NHC)
    hSv = hS.rearrange("p (hk s) -> p hk s", hk=NHC)
    hRv = [t.rearrange("p (hk t) -> p hk t", hk=NHC) for t in hR]
    xg_dv = xg_d.rearrange("p (k t) -> p k t", k=KC)
    wgu_dv = wgu_d.rearrange("p (j r k h) -> p j r k h", j=EPC, r=2, k=KC)
    xt_dv = xt_d.rearrange("p (k s) -> p k s", k=KC)

    # ---- DMA schedule. Transfers serialize per issuing queue (~330 GB/s
    # each in the model). Phase-1 expert-0 working set first, chunked to
    # match PE consumption order; expert-1 weights ride the scalar queue
    # (idle until the first psum results); the rest is balanced sync/pool
    # in order of first use.
    wgu_fv = wgu_d.rearrange("p (j r k h) -> p j r k h", j=EPC, r=2, k=KC)
    nc.sync.dma_start(wguv[:, 0, 0, :KC // 2], wgu_fv[:, 0, 0, :KC // 2])
    for k in range(KC):
        nc.gpsimd.dma_start(xgv[:, k, :caps[0]], xg_dv[:, k, :caps[0]])
    nc.sync.dma_start(wguv[:, 0, 0, KC // 2:], wgu_fv[:, 0, 0, KC // 2:])
    nc.sync.dma_start(wguv[:, 0, 1, :KC // 2], wgu_fv[:, 0, 1, :KC // 2])
    nc.sync.dma_start(wguv[:, 0, 1, KC // 2:], wgu_fv[:, 0, 1, KC // 2:])
    nc.scalar.dma_start(wguv[:, 1, 0], wgu_dv[:, 1, 0])
    nc.scalar.dma_start(wguv[:, 1, 1], wgu_dv[:, 1, 1])
    for k in range(KC):
        (nc.sync if k % 2 == 0 else nc.gpsimd).dma_start(
            xgv[:, k, caps[0]:], xg_dv[:, k, caps[0]:])
    nc.sync.dma_start(wsgu[:], wsgu_d[:])
    nc.gpsimd.dma_start(wdn[:], wdn_d[:])
    for k in range(KC):
        eng = nc.sync if k < KC // 2 else nc.gpsimd
        eng.dma_start(xtv[:, k], xt_dv[:, k])
    nc.gpsimd.dma_start(wsdn[:], wsdn_d[:])

    # Persistent PSUM pools spanning all phases (all 8 banks; no per-phase
    # scope-close barriers): pg/pu double-buffered for gate/up, po 4-deep
    # for the down projections.
    ppg = tc.alloc_tile_pool(name="ppg", bufs=2, space="PSUM")
    ppu = tc.alloc_tile_pool(name="ppu", bufs=2, space="PSUM")
    ppo = tc.alloc_tile_pool(name="ppo", bufs=4, space="PSUM")
    stmp = tc.alloc_tile_pool(name="stmp", bufs=3)
    sout = tc.alloc_tile_pool(name="sout", bufs=4)

    # ---------------- phase 1: routed gate/up -> hR ----------------
    for j in range(EPC):
        for hc in range(NHC):
            for (off, w) in chunks[j]:
                pg = ppg.tile([128, 512], F32, tag="pg")
                pu = ppu.tile([128, 512], F32, tag="pu")
                mv = [xgv[:, k, xoff[j] + off: xoff[j] + off + w]
                      for k in range(KC)]
                hsl = slice(hc * 128, (hc + 1) * 128)
                for k in range(KC):
                    nc.tensor.matmul(pg[:, :w], wguv[:, j, 0, k, hsl],
                                     mv[k],
                                     start=(k == 0), stop=(k == KC - 1))
                for k in range(KC):
                    nc.tensor.matmul(pu[:, :w], wguv[:, j, 1, k, hsl],
                                     mv[k],
                                     start=(k == 0), stop=(k == KC - 1))
                tmp = stmp.tile([128, 512], BF, tag="t1")
                nc.scalar.activation(tmp[:, :w], pg[:, :w], AF.Silu)
                nc.vector.tensor_mul(
                    hRv[j][:, hc, off:off + w],
                    tmp[:, :w], pu[:, :w])

    # ---------------- phase 2: routed down -> outR ----------------
    for cc in range(NCC):
        orr = sout.tile([128, tcap], BF, tag="or")
        cs = slice(cc * 128, (cc + 1) * 128)
        ci = 0
        for j in range(EPC):
            for (off, w) in chunks[j]:
                po = ppo.tile([128, 512], F32, tag="po")
                for hk in range(NHC):
                    nc.tensor.matmul(
                        po[:, :w], wdnv[:, j, hk, cs],
                        hRv[j][:, hk, off:off + w],
                        start=(hk == 0), stop=(hk == NHC - 1))
                dst = orr[:, xoff[j] + off: xoff[j] + off + w]
                if ci % 2 == 0:
                    nc.scalar.copy(dst, po[:, :w])
                else:
                    nc.vector.tensor_copy(dst, po[:, :w])
                ci += 1
        eng = nc.sync if cc % 2 == 0 else nc.gpsimd
        eng.dma_start(outR_d[cs, :], orr[:])

    # ---------------- phase 3: shared gate/up -> hS ----------------
    for hc in range(NHC):
        for sc in range(NSC):
            pg = ppg.tile([128, 512], F32, tag="pg")
            pu = ppu.tile([128, 512], F32, tag="pu")
            hsl = slice(hc * 128, (hc + 1) * 128)
            ss = slice(sc * 512, (sc + 1) * 512)
            for k in range(KC):
                nc.tensor.matmul(pg[:], wsguv[:, 0, k, hsl], xtv[:, k, ss],
                                 start=(k == 0), stop=(k == KC - 1))
            for k in range(KC):
                nc.tensor.matmul(pu[:], wsguv[:, 1, k, hsl], xtv[:, k, ss],
                                 start=(k == 0), stop=(k == KC - 1))
            tmp = stmp.tile([128, 512], BF, tag="t3")
            nc.scalar.activation(tmp[:], pg[:], AF.Silu)
            nc.vector.tensor_mul(hSv[:, hc, ss], tmp[:], pu[:])

    # ---------------- phase 4: shared down -> outS ----------------
    for cc in range(NCC):
        osr = sout.tile([128, S], BF, tag="os")
        cs = slice(cc * 128, (cc + 1) * 128)
        # the final row's last 512 columns are computed as two independent
        # 256-col chunks (own psum tiles / copy engines / DMA queues) so
        # the post-last-matmul tail is as short as possible
        chunks = ([(sc * 512, 512) for sc in range(NSC)] if cc < NCC - 1
                  else [(0, 512), (512, 512), (1024, 512),
                        (1536, 256), (1792, 256)])
        for i, (off, w) in enumerate(chunks):
            po = ppo.tile([128, 512], F32, tag="po")
            ss = slice(off, off + w)
            for hk in range(NHC):
                nc.tensor.matmul(po[:, :w], wsdnv[:, hk, cs], hSv[:, hk, ss],
                                 start=(hk == 0), stop=(hk == NHC - 1))
            if i % 2 == 0:
                nc.scalar.copy(osr[:, ss], po[:, :w])
            else:
                nc.vector.tensor_copy(osr[:, ss], po[:, :w])
            # stream the output row out as its chunks complete
            if cc < NCC - 1:
                if off == 512:
                    nc.sync.dma_start(outS_d[cs, :1024], osr[:, :1024])
                elif off == 1536:
                    nc.gpsimd.dma_start(outS_d[cs, 1024:], osr[:, 1024:])
            else:
                eng = nc.sync if i % 2 == 0 else nc.gpsimd
                eng.dma_start(outS_d[cs, ss], osr[:, ss])

    sout.release()
    stmp.release()
    ppo.release()
    ppu.release()
    ppg.release()
    res.release()


_NC_CACHE = {}


def _get_nc(caps):
    if caps not in _NC_CACHE:
        _NC_CACHE[caps] = build(caps)
    return _NC_CACHE[caps]


def _route_host(xf, router_w, correction_bias):
    """Replicates reference._route in float64 numpy (stable argsort matches
    jax.lax.top_k's lower-index-wins tie-breaking)."""
    x64 = xf.astype(np.float64)
    logits = x64 @ router_w.astype(np.float64).T           # [S, E]
    scores = 1.0 / (1.0 + np.exp(-logits))
    sb = scores + correction_bias.astype(np.float64)
    n = sb.shape[0]
    sbg = sb.reshape(n, G, EPG)
    grp_top = -np.sort(-sbg, axis=-1)[:, :, :PER_GROUP_K]
    group_scores = grp_top.sum(axis=-1)                    # [S, G]
    gidx = np.argsort(-group_scores, kind="stable", axis=-1)[:, :TOPK_GROUP]
    gmask = np.zeros((n, G))
    np.put_along_axis(gmask, gidx, 1.0, axis=-1)
    smask = np.repeat(gmask, EPG, axis=1)
    masked = np.where(smask > 0, sb, -np.inf)
    tk = np.argsort(-masked, kind="stable", axis=-1)[:, :TOPK]   # [S, K]
    wv = np.take_along_axis(scores, tk, axis=1)
    wv = wv / (wv.sum(axis=-1, keepdims=True) + 1e-20)
    return tk, wv


def _expert_token_lists(tk, wv):
    """Per expert: (token idx ascending, combine weights)."""
    out = []
    for e in range(E):
        tok, slot = np.nonzero(tk == e)
        out.append((tok, wv[tok, slot]))
    return out


def _assign_experts(experts):
    """(core, slot) -> expert id. Slot 0 takes the 8 most-loaded experts,
    slot 1 the rest, so slot 1's capacity (and its padding) is smaller."""
    order = np.argsort([-len(tok) for tok, _ in experts], kind="stable")
    assign = np.zeros((NCORES, EPC), np.int64)
    caps = []
    for j in range(EPC):
        grp = order[j * NCORES:(j + 1) * NCORES]
        assign[:, j] = grp
        cp = max(2, max(len(experts[e][0]) for e in grp))
        caps.append(cp + cp % 2)
    return assign, tuple(caps)


def _pack_contract(a):
    """[C_like, F] -> [128, (kc F)] with row index c = k*128 + p."""
    ck, f = a.shape
    kc = ck // 128
    return np.ascontiguousarray(
        a.reshape(kc, 128, f).transpose(1, 0, 2).reshape(128, kc * f))


def make_in_maps(x, router_w, correction_bias, gate_w, up_w, down_w,
                 shared_gate_w, shared_up_w, shared_down_w):
    x = np.asarray(x, dtype=np.float32)
    xf = x.reshape(S, C)
    tk, wv = _route_host(xf, np.asarray(router_w, np.float32),
                         np.asarray(correction_bias, np.float32))
    experts = _expert_token_lists(tk, wv)
    assign, caps = _assign_experts(experts)
    xoff = [0, caps[0]]
    tcap = caps[0] + caps[1]

    xT_bf = xf.T.astype(BF_NP)                              # [C, S]
    xt_pack = _pack_contract(xT_bf)                         # [128, KC*S]

    gate_w = np.asarray(gate_w, np.float32)
    up_w = np.asarray(up_w, np.float32)
    down_w = np.asarray(down_w, np.float32)
    sgT = np.asarray(shared_gate_w, np.float32).T           # [C, HS]
    suT = np.asarray(shared_up_w, np.float32).T             # [C, HS]
    sdT = np.asarray(shared_down_w, np.float32).T           # [HS, C]

    in_maps = []
    for c in range(NCORES):
        es = [int(assign[c, j]) for j in range(EPC)]
        hs = slice(c * HSL, (c + 1) * HSL)

        # gathered tokens [128, (k t)], slot layout [slot0 | slot1]
        xg = np.zeros((128, KC, tcap), BF_NP)
        for j, e in enumerate(es):
            tok, _w = experts[e]
            xsel = xf[tok].T.astype(BF_NP)                  # [C, n]
            xg[:, :, xoff[j]:xoff[j] + len(tok)] = (
                xsel.reshape(KC, 128, len(tok)).transpose(1, 0, 2))
        # routed gate/up [128, (j r k h)]
        wgu = np.stack(
            [np.stack([_pack_contract(gate_w[e].astype(BF_NP)),
                       _pack_contract(up_w[e].astype(BF_NP))], 1)
             for e in es], 1)                               # [128, j, 2, KC*H]
        # routed down [128, (j hk c)]
        wdn = np.stack([_pack_contract(down_w[e].astype(BF_NP))
                        for e in es], 1)
        wsgu = np.stack([_pack_contract(sgT[:, hs].astype(BF_NP)),
                         _pack_contract(suT[:, hs].astype(BF_NP))], 1)
        wsdn = _pack_contract(sdT[hs, :].astype(BF_NP))

        in_maps.append({
            "xt": xt_pack,
            "xg": np.ascontiguousarray(xg.reshape(128, KC * tcap)),
            "wgu": np.ascontiguousarray(wgu.reshape(128, -1)),
            "wdn": np.ascontiguousarray(wdn.reshape(128, -1)),
            "wsgu": np.ascontiguousarray(wsgu.reshape(128, -1)),
            "wsdn": np.ascontiguousarray(wsdn),
        })
    return in_maps, (experts, assign), caps


def postprocess(results, routing, caps):
    experts, assign = routing
    xoff = [0, caps[0]]
    accT = np.zeros((C, S), np.float64)
    for c in range(NCORES):
        accT += np.asarray(results[c]["outS"]).astype(np.float64)
        outR = np.asarray(results[c]["outR"]).astype(np.float64)
        for j in range(EPC):
            tok, w = experts[int(assign[c, j])]
            accT[:, tok] += (outR[:, xoff[j]: xoff[j] + len(tok)]
                             * w[None, :])
    return np.ascontiguousarray(accT.T).astype(np.float32).reshape(B, T, C)


def kernel(x, router_w, correction_bias, gate_w, up_w, down_w,
           shared_gate_w, shared_up_w, shared_down_w):
    in_maps, routing, caps = make_in_maps(
        x, router_w, correction_bias, gate_w, up_w, down_w,
        shared_gate_w, shared_up_w, shared_down_w)
    nc = _get_nc(caps)
    res = run_bass_kernel_spmd(nc, in_maps, list(range(NCORES)))
    return postprocess(res.results, routing, caps)


# revision 49
# speedup vs baseline: 1.0026x; 1.0026x over previous
"""MoE FFN (grouped sigmoid top-k routing + shared expert) on 8 TRN2 NeuronCores.

Strategy: expert-parallel with SPARSE token dispatch. Routing is computed on
the host (it determines the sharding itself — this harness's stand-in for the
"all-to-all token dispatch after routing" in the sharding hint): each core
gets 2 of 16 routed experts and receives only the tokens routed to them
(gathered + padded to a per-call capacity CAP per expert), plus 1/8 of the
shared expert (sharded along its hidden dim HS) over all tokens. All device
matmuls run in bf16 (rel-err budget 2e-2; bf16 lands ~5e-3), halving DMA and
SBUF versus fp32r at the same PE rate. Each core writes a dense shared-expert
partial [C, S] and its two experts' gathered outputs [C, CAP] (un-weighted);
the host applies the combine weights, scatter-adds, reduces over cores, and
transposes back.

Per-expert capacities are sized to the call's actual loads (slot 0 takes
the 8 most-loaded experts, slot 1 the rest, so slot 1's capacity and its
padding are smaller); no tokens are ever dropped, and the compiled kernel
is cached per capacity pair. Device work per core: (cap0+cap1)*3*C*H
(routed) + 3*S*C*HS/8 (shared) MACs ~ 2.4 G MACs -> ~62 us PE at 2.4 GHz
bf16, vs the dense-dispatch baseline's ~166 us.
"""

import numpy as np
import ml_dtypes

import concourse.bacc as bacc
import concourse.mybir as mybir
from concourse import tile
from concourse.bass_utils import run_bass_kernel_spmd

F32 = mybir.dt.float32
BF = mybir.dt.bfloat16
AF = mybir.ActivationFunctionType
BF_NP = ml_dtypes.bfloat16

# problem shapes (hardcoded; kernel.py must be self-contained)
B, T, C, H, HS = 2, 1024, 1024, 256, 2048
E, G, EPG = 16, 4, 4
TOPK, TOPK_GROUP = 4, 2
PER_GROUP_K = TOPK // TOPK_GROUP
NCORES = 8
S = B * T                  # 2048 tokens
EPC = E // NCORES          # 2 experts per core
HSL = HS // NCORES         # 256 shared-hidden rows per core
KC = C // 128              # 8 contraction chunks
NHC = H // 128             # 2 h chunks (same for HSL)
NSC = S // 512             # 4 moving (token) chunks of 512
NCC = C // 128             # 8 output-row chunks


def build(caps):
    cap0, cap1 = caps
    tcap = cap0 + cap1
    nc = bacc.Bacc(
        "TRN2",
        target_bir_lowering=False,
        debug=False,
        enable_asserts=True,
        num_devices=NCORES,
    )
    # ---- DRAM I/O (per core), all bf16, pre-packed [128, ...] on host ----
    xt_d = nc.declare_dram_parameter("xt", [128, KC * S], BF, isOutput=False)
    xg_d = nc.declare_dram_parameter("xg", [128, KC * tcap], BF,
                                     isOutput=False)
    wgu_d = nc.declare_dram_parameter("wgu", [128, EPC * 2 * KC * H], BF,
                                      isOutput=False)
    wdn_d = nc.declare_dram_parameter("wdn", [128, EPC * NHC * C], BF,
                                      isOutput=False)
    wsgu_d = nc.declare_dram_parameter("wsgu", [128, 2 * KC * HSL], BF,
                                       isOutput=False)
    wsdn_d = nc.declare_dram_parameter("wsdn", [128, NHC * C], BF,
                                       isOutput=False)
    outS_d = nc.declare_dram_parameter("outS", [C, S], BF, isOutput=True)
    outR_d = nc.declare_dram_parameter("outR", [C, tcap], BF,
                                       isOutput=True)

    with tile.TileContext(nc) as tc:
        _emit(nc, tc, caps, xt_d, xg_d, wgu_d, wdn_d, wsgu_d, wsdn_d,
              outS_d, outR_d)
    nc.finalize()
    return nc


def _emit(nc, tc, caps, xt_d, xg_d, wgu_d, wdn_d, wsgu_d, wsdn_d,
          outS_d, outR_d):
    # per-slot moving-chunk lists (each chunk must fit a 512-fp32 psum bank)
    xoff = [0, caps[0]]
    tcap = caps[0] + caps[1]
    chunks = []
    for cp in caps:
        n = -(-cp // 512)          # pieces of <=512 columns, even widths
        base = min(512, -(-cp // n) + (-(-cp // n)) % 2)
        pieces, off = [], 0
        while off < cp:
            w = min(base, cp - off)
            pieces.append((off, w))
            off += w
        chunks.append(pieces)
    res = tc.alloc_tile_pool(name="res", bufs=1)
    xg = res.tile([128, KC * tcap], BF)
    wgu = res.tile([128, EPC * 2 * KC * H], BF)
    wdn = res.tile([128, EPC * NHC * C], BF)
    xt = res.tile([128, KC * S], BF)
    wsgu = res.tile([128, 2 * KC * HSL], BF)
    wsdn = res.tile([128, NHC * C], BF)
    hS = res.tile([128, NHC * S], BF)
    hR = [res.tile([128, NHC * caps[j]], BF, name=f"hR{j}")
          for j in range(EPC)]

    xgv = xg.rearrange("p (k t) -> p k t", k=KC)
    wguv = wgu.rearrange("p (j r hc k h) -> p j r hc k h",
                         j=EPC, r=2, hc=NHC, k=KC)
    wdnv = wdn.rearrange("p (j hk c) -> p j hk c", j=EPC, hk=NHC)
    xtv = xt.rearrange("p (k s) -> p k s", k=KC)
    wsguv = wsgu.rearrange("p (r k h) -> p r k h", r=2, k=KC)
    wsdnv = wsdn.rearrange("p (hk c) -> p hk c", hk=NHC)
    hSv = hS.rearrange("p (hk s) -> p hk s", hk=NHC)
    hRv = [t.rearrange("p (hk t) -> p hk t", hk=NHC) for t in hR]
    xg_dv = xg_d.rearrange("p (k t) -> p k t", k=KC)
    wgu_dv = wgu_d.rearrange("p (j r hc k h) -> p j r hc k h",
                             j=EPC, r=2, hc=NHC, k=KC)
    xt_dv = xt_d.rearrange("p (k s) -> p k s", k=KC)

    # Warm the scalar engine's activation tables (Silu + Copy) with dummy
    # ops at t~0: the LoadActFuncSet (~1.3us) otherwise lands right when
    # the first gate psum needs silu, stalling the psum pipeline.
    warm = res.tile([128, 8], F32, name="warm")
    nc.vector.memset(warm[:], 0.0)
    nc.scalar.activation(warm[:], warm[:], AF.Silu)

    # ---- DMA schedule. Transfers serialize per issuing queue (~330 GB/s
    # each in the model). Phase-1 expert-0 working set first, chunked to
    # match PE consumption order; expert-1 weights ride the scalar queue
    # (idle until the first psum results); the rest is balanced sync/pool
    # in order of first use.
    nc.sync.dma_start(wguv[:, 0, 0, 0, :KC // 2], wgu_dv[:, 0, 0, 0, :KC // 2])
    for k in range(KC):
        nc.gpsimd.dma_start(xgv[:, k, :caps[0]], xg_dv[:, k, :caps[0]])
    nc.sync.dma_start(wguv[:, 0, 0, 0, KC // 2:], wgu_dv[:, 0, 0, 0, KC // 2:])
    nc.sync.dma_start(wguv[:, 0, 1, 0, :KC // 2], wgu_dv[:, 0, 1, 0, :KC // 2])
    nc.sync.dma_start(wguv[:, 0, 1, 0, KC // 2:], wgu_dv[:, 0, 1, 0, KC // 2:])
    nc.sync.dma_start(wguv[:, 0, 0, 1], wgu_dv[:, 0, 0, 1])
    nc.sync.dma_start(wguv[:, 0, 1, 1], wgu_dv[:, 0, 1, 1])
    nc.gpsimd.dma_start(wguv[:, 1], wgu_dv[:, 1])
    for k in range(KC):
        (nc.sync if k % 2 == 0 else nc.gpsimd).dma_start(
            xgv[:, k, caps[0]:], xg_dv[:, k, caps[0]:])
    nc.sync.dma_start(wsgu[:], wsgu_d[:])
    nc.gpsimd.dma_start(wdn[:], wdn_d[:])
    for k in range(KC):
        eng = nc.sync if k < 6 else nc.gpsimd
        eng.dma_start(xtv[:, k], xt_dv[:, k])
    nc.gpsimd.dma_start(wsdn[:], wsdn_d[:])

    # Persistent PSUM pools spanning all phases (all 8 banks; no per-phase
    # scope-close barriers): pg/pu double-buffered for gate/up, po 4-deep
    # for the down projections.
    ppg = tc.alloc_tile_pool(name="ppg", bufs=2, space="PSUM")
    ppu = tc.alloc_tile_pool(name="ppu", bufs=2, space="PSUM")
    ppo = tc.alloc_tile_pool(name="ppo", bufs=4, space="PSUM")
    stmp = tc.alloc_tile_pool(name="stmp", bufs=3)
    sout = tc.alloc_tile_pool(name="sout", bufs=4)

    # ---------------- phase 1: routed gate/up -> hR ----------------
    # alternate psum between ppg/ppu and the (idle-in-phase-1) ppo pool so
    # a section never waits on the silu/mul that drains the previous one
    p1i = 0
    for j in range(EPC):
        for hc in range(NHC):
            for (off, w) in chunks[j]:
                if p1i % 2 == 0:
                    pg = ppg.tile([128, 512], F32, tag="pg")
                    pu = ppu.tile([128, 512], F32, tag="pu")
                else:
                    pg = ppo.tile([128, 512], F32, tag="po")
                    pu = ppo.tile([128, 512], F32, tag="po")
                p1i += 1
                mv = [xgv[:, k, xoff[j] + off: xoff[j] + off + w]
                      for k in range(KC)]
                for k in range(KC):
                    nc.tensor.matmul(pg[:, :w], wguv[:, j, 0, hc, k], mv[k],
                                     start=(k == 0), stop=(k == KC - 1))
                for k in range(KC):
                    nc.tensor.matmul(pu[:, :w], wguv[:, j, 1, hc, k], mv[k],
                                     start=(k == 0), stop=(k == KC - 1))
                tmp = stmp.tile([128, 512], BF, tag="t1")
                nc.scalar.activation(tmp[:, :w], pg[:, :w], AF.Silu)
                nc.vector.tensor_mul(
                    hRv[j][:, hc, off:off + w],
                    tmp[:, :w], pu[:, :w])


    # ---------------- phase 2: routed down -> outR ----------------
    for cc in range(NCC):
        orr = sout.tile([128, tcap], BF, tag="or")
        cs = slice(cc * 128, (cc + 1) * 128)
        ci = 0
        for j in range(EPC):
            for (off, w) in chunks[j]:
                po = ppo.tile([128, 512], F32, tag="po")
                for hk in range(NHC):
                    nc.tensor.matmul(
                        po[:, :w], wdnv[:, j, hk, cs],
                        hRv[j][:, hk, off:off + w],
                        start=(hk == 0), stop=(hk == NHC - 1))
                dst = orr[:, xoff[j] + off: xoff[j] + off + w]
                if ci % 2 == 0:
                    nc.scalar.copy(dst, po[:, :w])
                else:
                    nc.vector.tensor_copy(dst, po[:, :w])
                ci += 1
        eng = nc.sync if cc % 2 == 0 else nc.gpsimd
        eng.dma_start(outR_d[cs, :], orr[:])

    # ---------------- phase 3: shared gate/up -> hS ----------------
    for hc in range(NHC):
        for sc in range(NSC):
            pg = ppg.tile([128, 512], F32, tag="pg")
            pu = ppu.tile([128, 512], F32, tag="pu")
            hsl = slice(hc * 128, (hc + 1) * 128)
            ss = slice(sc * 512, (sc + 1) * 512)
            for k in range(KC):
                nc.tensor.matmul(pg[:], wsguv[:, 0, k, hsl], xtv[:, k, ss],
                                 start=(k == 0), stop=(k == KC - 1))
            for k in range(KC):
                nc.tensor.matmul(pu[:], wsguv[:, 1, k, hsl], xtv[:, k, ss],
                                 start=(k == 0), stop=(k == KC - 1))
            tmp = stmp.tile([128, 512], BF, tag="t3")
            nc.scalar.activation(tmp[:], pg[:], AF.Silu)
            nc.vector.tensor_mul(hSv[:, hc, ss], tmp[:], pu[:])

    # ---------------- phase 4: shared down -> outS ----------------
    for cc in range(NCC):
        osr = sout.tile([128, S], BF, tag="os")
        cs = slice(cc * 128, (cc + 1) * 128)
        # the final row's last 512 columns are computed as two independent
        # 256-col chunks (own psum tiles / copy engines / DMA queues) so
        # the post-last-matmul tail is as short as possible
        chunks = ([(sc * 512, 512) for sc in range(NSC)] if cc < NCC - 1
                  else [(0, 512), (512, 512), (1024, 512),
                        (1536, 256), (1792, 256)])
        for i, (off, w) in enumerate(chunks):
            po = ppo.tile([128, 512], F32, tag="po")
            ss = slice(off, off + w)
            for hk in range(NHC):
                nc.tensor.matmul(po[:, :w], wsdnv[:, hk, cs], hSv[:, hk, ss],
                                 start=(hk == 0), stop=(hk == NHC - 1))
            if i % 2 == 0:
                nc.scalar.copy(osr[:, ss], po[:, :w])
            else:
                nc.vector.tensor_copy(osr[:, ss], po[:, :w])
            # stream the output row out as its chunks complete
            if cc < NCC - 1:
                if off == 512:
                    nc.sync.dma_start(outS_d[cs, :1024], osr[:, :1024])
                elif off == 1536:
                    nc.gpsimd.dma_start(outS_d[cs, 1024:], osr[:, 1024:])
            else:
                eng = nc.sync if i % 2 == 0 else nc.gpsimd
                eng.dma_start(outS_d[cs, ss], osr[:, ss])

    sout.release()
    stmp.release()
    ppo.release()
    ppu.release()
    ppg.release()
    res.release()


_NC_CACHE = {}


def _get_nc(caps):
    if caps not in _NC_CACHE:
        _NC_CACHE[caps] = build(caps)
    return _NC_CACHE[caps]


def _route_host(xf, router_w, correction_bias):
    """Replicates reference._route in float64 numpy (stable argsort matches
    jax.lax.top_k's lower-index-wins tie-breaking)."""
    x64 = xf.astype(np.float64)
    logits = x64 @ router_w.astype(np.float64).T           # [S, E]
    scores = 1.0 / (1.0 + np.exp(-logits))
    sb = scores + correction_bias.astype(np.float64)
    n = sb.shape[0]
    sbg = sb.reshape(n, G, EPG)
    grp_top = -np.sort(-sbg, axis=-1)[:, :, :PER_GROUP_K]
    group_scores = grp_top.sum(axis=-1)                    # [S, G]
    gidx = np.argsort(-group_scores, kind="stable", axis=-1)[:, :TOPK_GROUP]
    gmask = np.zeros((n, G))
    np.put_along_axis(gmask, gidx, 1.0, axis=-1)
    smask = np.repeat(gmask, EPG, axis=1)
    masked = np.where(smask > 0, sb, -np.inf)
    tk = np.argsort(-masked, kind="stable", axis=-1)[:, :TOPK]   # [S, K]
    wv = np.take_along_axis(scores, tk, axis=1)
    wv = wv / (wv.sum(axis=-1, keepdims=True) + 1e-20)
    return tk, wv


def _expert_token_lists(tk, wv):
    """Per expert: (token idx ascending, combine weights)."""
    out = []
    for e in range(E):
        tok, slot = np.nonzero(tk == e)
        out.append((tok, wv[tok, slot]))
    return out


def _assign_experts(experts):
    """(core, slot) -> expert id. Slot 0 takes the 8 most-loaded experts,
    slot 1 the rest, so slot 1's capacity (and its padding) is smaller."""
    order = np.argsort([-len(tok) for tok, _ in experts], kind="stable")
    assign = np.zeros((NCORES, EPC), np.int64)
    caps = []
    for j in range(EPC):
        grp = order[j * NCORES:(j + 1) * NCORES]
        assign[:, j] = grp
        cp = max(2, max(len(experts[e][0]) for e in grp))
        caps.append(cp + cp % 2)
    return assign, tuple(caps)


def _pack_contract(a):
    """[C_like, F] -> [128, (kc F)] with row index c = k*128 + p."""
    ck, f = a.shape
    kc = ck // 128
    return np.ascontiguousarray(
        a.reshape(kc, 128, f).transpose(1, 0, 2).reshape(128, kc * f))


def make_in_maps(x, router_w, correction_bias, gate_w, up_w, down_w,
                 shared_gate_w, shared_up_w, shared_down_w):
    x = np.asarray(x, dtype=np.float32)
    xf = x.reshape(S, C)
    tk, wv = _route_host(xf, np.asarray(router_w, np.float32),
                         np.asarray(correction_bias, np.float32))
    experts = _expert_token_lists(tk, wv)
    assign, caps = _assign_experts(experts)
    xoff = [0, caps[0]]
    tcap = caps[0] + caps[1]

    xT_bf = xf.T.astype(BF_NP)                              # [C, S]
    xt_pack = _pack_contract(xT_bf)                         # [128, KC*S]

    gate_w = np.asarray(gate_w, np.float32)
    up_w = np.asarray(up_w, np.float32)
    down_w = np.asarray(down_w, np.float32)
    sgT = np.asarray(shared_gate_w, np.float32).T           # [C, HS]
    suT = np.asarray(shared_up_w, np.float32).T             # [C, HS]
    sdT = np.asarray(shared_down_w, np.float32).T           # [HS, C]

    in_maps = []
    for c in range(NCORES):
        es = [int(assign[c, j]) for j in range(EPC)]
        hs = slice(c * HSL, (c + 1) * HSL)

        # gathered tokens [128, (k t)], slot layout [slot0 | slot1]
        xg = np.zeros((128, KC, tcap), BF_NP)
        for j, e in enumerate(es):
            tok, _w = experts[e]
            xsel = xf[tok].T.astype(BF_NP)                  # [C, n]
            xg[:, :, xoff[j]:xoff[j] + len(tok)] = (
                xsel.reshape(KC, 128, len(tok)).transpose(1, 0, 2))
        # routed gate/up [128, (j r hc k h)]: hc-major so the h-half the
        # PE consumes first can land first
        def _pack_gu(wmat):
            a = wmat.astype(BF_NP).reshape(KC, 128, NHC, 128)
            return a.transpose(1, 2, 0, 3).reshape(128, NHC * KC * 128)
        wgu = np.stack(
            [np.stack([_pack_gu(gate_w[e]), _pack_gu(up_w[e])], 1)
             for e in es], 1)                               # [128, j, 2, ...]
        # routed down [128, (j hk c)]
        wdn = np.stack([_pack_contract(down_w[e].astype(BF_NP))
                        for e in es], 1)
        wsgu = np.stack([_pack_contract(sgT[:, hs].astype(BF_NP)),
                         _pack_contract(suT[:, hs].astype(BF_NP))], 1)
        wsdn = _pack_contract(sdT[hs, :].astype(BF_NP))

        in_maps.append({
            "xt": xt_pack,
            "xg": np.ascontiguousarray(xg.reshape(128, KC * tcap)),
            "wgu": np.ascontiguousarray(wgu.reshape(128, -1)),
            "wdn": np.ascontiguousarray(wdn.reshape(128, -1)),
            "wsgu": np.ascontiguousarray(wsgu.reshape(128, -1)),
            "wsdn": np.ascontiguousarray(wsdn),
        })
    return in_maps, (experts, assign), caps


def postprocess(results, routing, caps):
    experts, assign = routing
    xoff = [0, caps[0]]
    accT = np.zeros((C, S), np.float64)
    for c in range(NCORES):
        accT += np.asarray(results[c]["outS"]).astype(np.float64)
        outR = np.asarray(results[c]["outR"]).astype(np.float64)
        for j in range(EPC):
            tok, w = experts[int(assign[c, j])]
            accT[:, tok] += (outR[:, xoff[j]: xoff[j] + len(tok)]
                             * w[None, :])
    return np.ascontiguousarray(accT.T).astype(np.float32).reshape(B, T, C)


def kernel(x, router_w, correction_bias, gate_w, up_w, down_w,
           shared_gate_w, shared_up_w, shared_down_w):
    in_maps, routing, caps = make_in_maps(
        x, router_w, correction_bias, gate_w, up_w, down_w,
        shared_gate_w, shared_up_w, shared_down_w)
    nc = _get_nc(caps)
    res = run_bass_kernel_spmd(nc, in_maps, list(range(NCORES)))
    return postprocess(res.results, routing, caps)
